# revision 1
# baseline (speedup 1.0000x reference)
"""Causal self-attention (B=2, T=2048, C=1024, 16 heads) on 8 trn2 cores.

Sharding: core = (batch b, head-group hg) on a 2x4 grid; each core computes
QKV projection, causal attention and the partial c_proj for its 4 heads of
one batch element. Host sums the 4 partials per batch element (replaces the
all-reduce) and adds bproj + bv@Wproj (the V-bias contribution is exact
because softmax rows sum to 1).

Device layout per core (all matmuls bf16):
  - x arrives host-transposed as xT [C=1024, T=2048], loaded in column
    chunks so the first projection matmul starts early.
  - Q^T/K^T computed as [qkv_col, t] tiles (head h lives at partitions
    (h%2)*64..) via matmul(lhsT=W_slice, rhs=xT).
  - V stored [t, per-head [V(64)|1|zeros]] so AV's psum row 64 accumulates
    the softmax denominator.
  - Scores transposed: S^T[k, q] = matmul(lhsT=K^T_ktile, rhs=Q^T_qblock)
    into [128,1024] PSUM pair tiles; exp on ScalarE (one merged instruction
    per full pair); causal masking applied AFTER exp by gpsimd
    affine_select zeroing the upper triangle of diagonal windows in SBUF,
    off the PSUM critical path.
  - AV accumulates O^T[d, q] over k-tiles; O rows 0..64 copied to SBUF
    right after the last AV to free the PSUM bank early; denominator folded
    to [128,4] via DRAM, exact reciprocal on DVE, broadcast back via DRAM;
    y^T = O^T * r.
  - Partial c_proj DMA'd directly from PSUM to HBM; proj work for q-block
    N is interleaved into q-block N-1's attention to fill PE bubbles while
    the scalar engine (exp) is the local bottleneck.
  - q-blocks processed 3,2,1,0 so the shortest dependency chain is last.
"""

import sys
import types

import numpy as np

# ---------------------------------------------------------------------------
# Environment compatibility (self-contained on purpose).
# ---------------------------------------------------------------------------


def _install_axon_ntff_hook():
    """Provide the missing ``antenv.axon_hooks`` module so that
    ``run_bass_kernel_spmd(trace=True)`` works under axon in this container."""
    if "antenv.axon_hooks" in sys.modules:
        return
    try:
        import antenv
    except ImportError:
        return
    mod = types.ModuleType("antenv.axon_hooks")
    holder = [None]
    mod.set_axon_ntff_profile_hook = lambda h: holder.__setitem__(0, h)
    mod.get_axon_ntff_profile_hook = lambda: holder[0]
    sys.modules["antenv.axon_hooks"] = mod
    antenv.axon_hooks = mod
    try:
        from trn_agent_boot.trn_boot import _ntff_profile_via_ctypes

        hook = _ntff_profile_via_ctypes("/opt/axon/libaxon_pjrt.so")
        if hook is not None:
            mod.set_axon_ntff_profile_hook(hook)
    except Exception:
        pass


_install_axon_ntff_hook()

import concourse.bass as bass  # noqa: E402
import concourse.mybir as mybir  # noqa: E402
import concourse.tile as tile  # noqa: E402
from concourse.bass_utils import run_bass_kernel_spmd  # noqa: E402


def _split_multi_waits(nc, max_waits=1):
    """The walrus build here rejects instructions with more than one sync
    wait; move excess waits onto same-engine NoOps placed just before the
    instruction (sequential waiting is equivalent for monotonic sems)."""
    n = 0
    for func in nc.m.functions:
        for bb in func.blocks:
            out = []
            changed = False
            for inst in bb.instructions:
                si = inst.sync_info
                waits = list(si.on_wait) if si is not None and si.on_wait else []
                if len(waits) > max_waits:
                    changed = True
                    extra, keep = waits[:-max_waits], waits[-max_waits:]
                    for i in range(0, len(extra), max_waits):
                        n += 1
                        out.append(
                            mybir.InstNoOp(
                                name=f"{inst.name}-ws{i}",
                                engine=inst.engine,
                                ins=[],
                                outs=[],
                                sync_info=mybir.SyncInfo(
                                    on_wait=extra[i : i + max_waits], on_update=[]
                                ),
                                text_hint="wait_split",
                            )
                        )
                    si.on_wait = keep
                out.append(inst)
            if changed:
                bb.instructions = out
    return n


# ---------------------------------------------------------------------------
# Problem constants (hardcoded per spec).
# ---------------------------------------------------------------------------

B, T, C = 2, 2048, 1024
N_HEAD = 16
D = 64  # head dim
N_CORES = 8
HG = 4  # head groups (cores per batch element)
NH = N_HEAD // HG  # heads per core = 4
HD = NH * D  # head channels per core = 256
CK = C // 128  # contraction chunks = 8
TT = T // 128  # t tiles = 16
QB = T // 512  # q blocks = 4

F32 = mybir.dt.float32
MM_DT = mybir.dt.bfloat16
MM_NP = mybir.dt.np(MM_DT)

TRACE = False
LAST_RESULT = None
_NC_CACHE = {}


def _build_nc():
    nc = bass.Bass("TRN2", target_bir_lowering=False)

    xT = nc.dram_tensor("xT", [C, T], MM_DT, kind="ExternalInput")
    wq = nc.dram_tensor("wq", [C, HD], MM_DT, kind="ExternalInput")
    wk = nc.dram_tensor("wk", [C, HD], MM_DT, kind="ExternalInput")
    wv = nc.dram_tensor("wv", [C, HD], MM_DT, kind="ExternalInput")
    bq = nc.dram_tensor("bq", [128, NH], F32, kind="ExternalInput")
    bk = nc.dram_tensor("bk", [128, HD // 128], F32, kind="ExternalInput")
    wp = nc.dram_tensor("wp", [HD, C], MM_DT, kind="ExternalInput")
    out = nc.dram_tensor("out", [T, C], F32, kind="ExternalOutput")

    with tile.TileContext(nc) as tc:
        _emit(nc, tc, xT, wq, wk, wv, bq, bk, wp, out)

    _split_multi_waits(nc)
    return nc


def _emit(nc, tc, xT, wq, wk, wv, bq, bk, wp, out):
    from contextlib import ExitStack

    ctx = ExitStack()
    with ctx:
        consts = ctx.enter_context(tc.tile_pool(name="consts", bufs=1))
        xt_pool = ctx.enter_context(tc.tile_pool(name="xt", bufs=CK))
        qz_pool = ctx.enter_context(tc.tile_pool(name="qz", bufs=NH))
        kt_pool = ctx.enter_context(tc.tile_pool(name="kt", bufs=HD // 128))
        vo_pool = ctx.enter_context(tc.tile_pool(name="vo", bufs=TT))
        yt_pool = ctx.enter_context(tc.tile_pool(name="yt", bufs=2))
        pt_pool = ctx.enter_context(tc.tile_pool(name="pt", bufs=6))
        os_pool = ctx.enter_context(tc.tile_pool(name="os", bufs=3))
        rb_pool = ctx.enter_context(tc.tile_pool(name="rb", bufs=3))
        dram = ctx.enter_context(tc.tile_pool(name="dram", bufs=3, space="DRAM"))
        # PSUM: st pairs 2x2 banks + ot 2 + qk 2 = 8 banks
        p_qk = ctx.enter_context(tc.tile_pool(name="p_qk", bufs=2, space="PSUM"))
        p_st = ctx.enter_context(tc.tile_pool(name="p_st", bufs=2, space="PSUM"))
        p_ot = ctx.enter_context(tc.tile_pool(name="p_ot", bufs=2, space="PSUM"))

        # ---- constant loads -------------------------------------------------
        bk_sb = consts.tile([128, HD // 128], F32, tag="bk")
        nc.sync.dma_start(bk_sb[:], bk[:])
        bq_sb = consts.tile([128, NH], F32, tag="bq")
        nc.sync.dma_start(bq_sb[:], bq[:])

        # wv/wk loaded in per-ck pieces so the first V/K chains only wait for
        # ~64KB per contraction step.
        wv_t = consts.tile([128, CK, HD], MM_DT, tag="wv")
        wv_r = wv.rearrange("(o p) n -> p o n", p=128)
        wk_t = consts.tile([128, CK, HD], MM_DT, tag="wk")
        wk_r = wk.rearrange("(o p) n -> p o n", p=128)
        for ck in range(CK):
            nc.sync.dma_start(wv_t[:, ck : ck + 1], wv_r[:, ck : ck + 1])
        w_sb = {"wv": wv_t, "wk": wk_t}

        # x column chunks (256 cols x all ck): V tiles for a chunk start as
        # soon as it lands.
        xt_sb = [
            xt_pool.tile([128, T], MM_DT, tag="xt", name=f"xt{ck}")
            for ck in range(CK)
        ]

        def load_x_chunk(chunk):
            cs = slice(chunk * 256, (chunk + 1) * 256)
            for ck in range(CK):
                nc.sync.dma_start(
                    xt_sb[ck][:, cs], xT[ck * 128 : (ck + 1) * 128, cs]
                )

        load_x_chunk(0)
        for ck in range(CK):
            nc.sync.dma_start(wk_t[:, ck : ck + 1], wk_r[:, ck : ck + 1])
        load_x_chunk(1)
        wq_t = consts.tile([128, CK, HD], MM_DT, tag="wq")
        nc.sync.dma_start(wq_t[:], wq.rearrange("(o p) n -> p o n", p=128))
        w_sb["wq"] = wq_t
        load_x_chunk(2)
        wp_sb = consts.tile([128, HD // 128, C], MM_DT, tag="wp")
        nc.sync.dma_start(wp_sb[:], wp.rearrange("(o p) n -> p o n", p=128))
        for chunk in range(3, 8):
            load_x_chunk(chunk)

        # ---- QKV projection -------------------------------------------------
        qz_sb = [
            qz_pool.tile([128, T], MM_DT, tag="qz", name=f"qz{h}") for h in range(NH)
        ]
        kt_sb = [
            kt_pool.tile([128, T], MM_DT, tag="kt", name=f"kt{i}")
            for i in range(HD // 128)
        ]
        for h in range(NH):
            zb = 64 - (h % 2) * 64  # zero the half NOT holding Q_h
            nc.gpsimd.memset(qz_sb[h][zb : zb + 64, :], 0.0)

        def emit_k(i, tb):
            tbc = slice(tb * 512, (tb + 1) * 512)
            ps = p_qk.tile([128, 512], F32, tag="pq")
            for ck in range(CK):
                nc.tensor.matmul(
                    ps[:],
                    w_sb["wk"][:, ck, i * 128 : (i + 1) * 128],
                    xt_sb[ck][:, tbc],
                    start=(ck == 0),
                    stop=(ck == CK - 1),
                )
            nc.vector.tensor_scalar(
                kt_sb[i][:, tbc],
                ps[:],
                bk_sb[:, i : i + 1],
                None,
                mybir.AluOpType.add,
            )

        def emit_q(i, tb):
            tbc = slice(tb * 512, (tb + 1) * 512)
            ps = p_qk.tile([128, 512], F32, tag="pq")
            for ck in range(CK):
                nc.tensor.matmul(
                    ps[:],
                    w_sb["wq"][:, ck, i * 128 : (i + 1) * 128],
                    xt_sb[ck][:, tbc],
                    start=(ck == 0),
                    stop=(ck == CK - 1),
                )
            for hh in (2 * i, 2 * i + 1):
                hb = (hh % 2) * 64
                nc.vector.tensor_scalar(
                    qz_sb[hh][hb : hb + 64, tbc],
                    ps[hb : hb + 64, :],
                    bq_sb[hb : hb + 64, hh : hh + 1],
                    None,
                    mybir.AluOpType.add,
                )

        # V tiles [128, NH*128]: per head [V(64) | ones | zeros(63)] so the
        # AV matmul's psum row 64 accumulates the softmax denominator.
        vo_sb = [None] * TT

        def emit_v(tt):
            t = vo_pool.tile([128, NH * 128], MM_DT, tag="vo", name=f"vo{tt}")
            vo_sb[tt] = t
            v4 = t[:].rearrange("p (h c) -> p h c", h=NH)
            nc.gpsimd.memset(v4[:, :, D + 1 :], 0.0)
            nc.gpsimd.memset(v4[:, :, D : D + 1], 1.0)
            ps = p_qk.tile([128, 512], F32, tag="pq")
            for ck in range(CK):
                nc.tensor.matmul(
                    ps[:, :HD],
                    xt_sb[ck][:, tt * 128 : (tt + 1) * 128],
                    w_sb["wv"][:, ck, :],
                    start=(ck == 0),
                    stop=(ck == CK - 1),
                )
            nc.vector.tensor_copy(
                v4[:, :, 0:D],
                ps[:, :HD].rearrange("p (h c) -> p h c", h=NH),
            )

        # V tiles first (each needs only a 128-col x slice), K interleaved
        # as its 512-col spans complete.
        for chunk in range(8):
            emit_v(2 * chunk)
            emit_v(2 * chunk + 1)
            if chunk % 2 == 1:
                tb = chunk // 2
                emit_k(0, tb)
                emit_k(1, tb)

        # ---- attention ------------------------------------------------------
        yt_sb = [
            yt_pool.tile([128, T], MM_DT, tag="yt", name=f"yt{g}")
            for g in range(HD // 128)
        ]

        # c_proj granules: one PSUM tile (2 matmuls + SBUF bounce + DMA)
        # each; queued when a q-block's heads finish, drained as PE fillers
        # while the next q-block's attention is exp(Act)-bound.
        proj_queue = []
        ob_pool = ctx.enter_context(tc.tile_pool(name="ob", bufs=3))

        def make_proj(tt, nb):
            def emit_proj():
                ps = p_qk.tile([128, 512], F32, tag="pq")
                for g in range(HD // 128):
                    nc.tensor.matmul(
                        ps[:],
                        yt_sb[g][:, tt * 128 : (tt + 1) * 128],
                        wp_sb[:, g, nb * 512 : (nb + 1) * 512],
                        start=(g == 0),
                        stop=(g == HD // 128 - 1),
                    )
                ob = ob_pool.tile([128, 512], F32, tag="ob")
                nc.vector.tensor_copy(ob[:], ps[:])
                nc.sync.dma_start(
                    out[tt * 128 : (tt + 1) * 128, nb * 512 : (nb + 1) * 512],
                    ob[:],
                )

            return emit_proj

        def drain_proj(n):
            for _ in range(min(n, len(proj_queue))):
                proj_queue.pop(0)()

        def emit_attention(qb):
            q0 = qb * 512
            n_kt = 4 * qb + 4
            n_pair = n_kt // 2

            def emit_pair(h, pi):
                """Score matmuls + exp for k-tiles (2pi, 2pi+1) into one
                [128,1024] PSUM pair; returns (pt_tile, col offsets)."""
                kd = kt_sb[h // 2]
                qd = qz_sb[h]
                st = p_st.tile([128, 1024], F32, tag="st")
                pt = pt_pool.tile([128, 1024], MM_DT, tag="pt")
                cols = []
                for jj in range(2):
                    kt = 2 * pi + jj
                    j = kt - 4 * qb
                    if j < 0:
                        c = 0
                    elif j < 3:
                        c = 128 * j
                    else:
                        c = 384
                    cols.append(c)
                    nc.tensor.matmul(
                        st[:, jj * 512 + c : jj * 512 + 512],
                        kd[:, kt * 128 : (kt + 1) * 128],
                        qd[:, q0 + c : q0 + 512],
                        start=True,
                        stop=True,
                    )
                if cols[0] == 0 and cols[1] == 0:
                    # full pair: one merged exp over both halves
                    nc.scalar.activation(
                        pt[:, 0:1024],
                        st[:, 0:1024],
                        mybir.ActivationFunctionType.Exp,
                        scale=0.125,
                    )
                else:
                    for jj in range(2):
                        c = cols[jj]
                        nc.scalar.activation(
                            pt[:, jj * 512 + c : jj * 512 + 512],
                            st[:, jj * 512 + c : jj * 512 + 512],
                            mybir.ActivationFunctionType.Exp,
                            scale=0.125,
                        )
                # causal mask: zero the upper triangle of diagonal
                # 128-col windows, after exp, on gpsimd (SBUF).
                for jj in range(2):
                    kt = 2 * pi + jj
                    j = kt - 4 * qb
                    if j >= 0:
                        w0 = jj * 512 + 128 * j
                        nc.gpsimd.affine_select(
                            out=pt[:, w0 : w0 + 128],
                            in_=pt[:, w0 : w0 + 128],
                            compare_op=mybir.AluOpType.is_ge,
                            fill=0.0,
                            base=0,
                            pattern=[[1, 128]],
                            channel_multiplier=-1,
                        )
                return pt, cols

            def normalize(h, ot):
                # early PSUM release: O^T rows 0..64 (out + denominator)
                # to SBUF f32, freeing the ot bank; then denominator folded
                # [1,512] -> [128,4] via DRAM, exact reciprocal on DVE,
                # broadcast back via DRAM; y^T = O^T * r.
                i, jb = h // 2, (h % 2) * 64
                o_sb = os_pool.tile([65, 512], F32, tag="os")
                nc.vector.tensor_copy(o_sb[:], ot[0:65, :])
                rc_d = dram.tile([1, 512], F32, tag="rc_d")
                nc.sync.dma_start(rc_d[:], o_sb[64:65, :])
                r4 = rb_pool.tile([128, 4], F32, tag="r4")
                nc.sync.dma_start(r4[:], rc_d[0, :].rearrange("(p o) -> p o", p=128))
                nc.vector.reciprocal(r4[:], r4[:])
                rc2_d = dram.tile([1, 512], F32, tag="rc2_d")
                nc.sync.dma_start(
                    rc2_d[0, :].rearrange("(p o) -> p o", p=128), r4[:]
                )
                rb = rb_pool.tile([64, 512], F32, tag="rb")
                nc.sync.dma_start(rb[:], rc2_d[:].to_broadcast((64, 512)))
                nc.vector.tensor_tensor(
                    yt_sb[i][jb : jb + 64, q0 : q0 + 512],
                    o_sb[0:64, :],
                    rb[:],
                    mybir.AluOpType.mult,
                )

            # two heads interleaved per pass: the exp conveyor on the scalar
            # engine never drains at head boundaries, and each head's AV lag
            # is covered by the other head's score matmuls.
            for hp in range(NH // 2):
                hs = (2 * hp, 2 * hp + 1)
                ots = {
                    h: p_ot.tile([128, 512], F32, tag="ot", name=f"ot{h}")
                    for h in hs
                }

                def emit_av(h, pi, pt, cols):
                    for jj in range(2):
                        kt = 2 * pi + jj
                        c = cols[jj]
                        nc.tensor.matmul(
                            ots[h][:, c:512],
                            vo_sb[kt][:, h * 128 : (h + 1) * 128],
                            pt[:, jj * 512 + c : jj * 512 + 512],
                            start=(kt == 0),
                            stop=(kt == n_kt - 1),
                        )

                pending = {h: [] for h in hs}
                for pi in range(n_pair):
                    for h in hs:
                        pt, cols = emit_pair(h, pi)
                        pending[h].append((h, pi, pt, cols))
                        if len(pending[h]) > 1:
                            emit_av(*pending[h].pop(0))
                    drain_proj(1 if qb >= 2 else 2)
                for h in hs:
                    for p in pending[h]:
                        emit_av(*p)
                    normalize(h, ots[h])
                if qb == 0:
                    drain_proj(4)

            # queue this q-block's c_proj granules (drained during the next
            # q-block's attention; leftovers drained after the loop).
            for tt in range(qb * 4, qb * 4 + 4):
                for nb in range(C // 512):
                    proj_queue.append(make_proj(tt, nb))

        # q-blocks descending: longest k-chain first, shortest last (small
        # serial tail). Q projection for each block emitted just before it.
        for qb in (3, 2, 1, 0):
            emit_q(0, qb)
            emit_q(1, qb)
            emit_attention(qb)
        drain_proj(len(proj_queue))


def _get_nc():
    key = str(MM_DT)
    if key not in _NC_CACHE:
        _NC_CACHE[key] = _build_nc()
    return _NC_CACHE[key]


def _dup_bias(b):
    # [NH*64] -> [128, NH]: head h's 64 biases replicated on both halves
    m = b.reshape(NH, 64).T  # [64, NH]
    return np.ascontiguousarray(np.vstack([m, m]).astype(np.float32))


def kernel(x, Wqkv, bqkv, Wproj, bproj):
    global LAST_RESULT
    x = np.asarray(x, dtype=np.float32)
    Wqkv = np.asarray(Wqkv, dtype=np.float32)
    bqkv = np.asarray(bqkv, dtype=np.float32)
    Wproj = np.asarray(Wproj, dtype=np.float32)
    bproj = np.asarray(bproj, dtype=np.float32)

    nc = _get_nc()
    in_maps = []
    for core in range(N_CORES):
        b, hg = core // HG, core % HG
        cs, ce = hg * HD, (hg + 1) * HD
        in_maps.append(
            {
                "xT": np.ascontiguousarray(x[b].T.astype(MM_NP)),
                "wq": np.ascontiguousarray(Wqkv[:, cs:ce].astype(MM_NP)),
                "wk": np.ascontiguousarray(Wqkv[:, C + cs : C + ce].astype(MM_NP)),
                "wv": np.ascontiguousarray(
                    Wqkv[:, 2 * C + cs : 2 * C + ce].astype(MM_NP)
                ),
                "bq": _dup_bias(bqkv[cs:ce]),
                "bk": np.ascontiguousarray(
                    bqkv[C + cs : C + ce].reshape(2, 128).T.astype(np.float32)
                ),
                "wp": np.ascontiguousarray(Wproj[cs:ce, :].astype(MM_NP)),
            }
        )

    res = run_bass_kernel_spmd(
        nc, in_maps, core_ids=list(range(N_CORES)), trace=TRACE
    )
    LAST_RESULT = res

    # V-bias contribution: y_true = y_dev + bv per head concat, and softmax
    # rows sum to exactly 1, so out += bv @ Wproj (host-side, exact).
    bv_full = bqkv[2 * C : 3 * C]
    bias_term = bv_full @ Wproj + bproj

    outp = np.empty((B, T, C), dtype=np.float32)
    for b in range(B):
        acc = res.results[b * HG]["out"].astype(np.float32).copy()
        for hg in range(1, HG):
            acc += res.results[b * HG + hg]["out"]
        outp[b] = acc + bias_term
    return outp



# revision 8
# speedup vs baseline: 1.0631x; 1.0631x over previous
"""Causal self-attention (B=2, T=2048, C=1024, 16 heads) on 8 trn2 cores.

Sharding: core = (batch b, head-group hg) on a 2x4 grid; each core computes
QKV projection, causal attention and the partial c_proj for its 4 heads of
one batch element. Host sums the 4 partials per batch element (replaces the
all-reduce) and adds bproj + bv@Wproj (the V-bias contribution is exact
because softmax rows sum to 1).

Device layout per core (all matmuls bf16):
  - x arrives host-prearranged chunk-major as xh [128, chunk=8, ck=8, 256]
    so each 256-column chunk is ONE dma (128 descriptors) and the first
    projection matmuls start ~2us in.  Weights arrive p-major so each is a
    single 128-descriptor dma.  DMA issue is split across sync (x, out,
    broadcasts) and vector (weights) queues.
  - K^T / Q^T produced as [128, T] bf16 tiles holding a HEAD-PAIR: head
    2hp at partitions 0..63, head 2hp+1 at 64..127.  One [128,512]
    tensor_scalar adds the bias (per-partition AP) and casts to bf16.
  - Scores: per k-tile, BOTH heads of a pair via two K=64 matmuls on
    disjoint PE row-groups (tile_position (0,0) / (64,0)) into one
    [128,1024] PSUM pair; they execute concurrently on the 128x128 array.
  - exp: ONE ScalarE activation per k-tile over [128, 2, 512-c] (both
    heads, ragged diagonal offset c shared).  Causal mask applied AFTER
    exp by one gpsimd affine_select per diagonal k-tile (both heads).
  - V stored [128, tt, h, 65] = per head [V(64) | 1]; AV accumulates
    O^T[65, q] so PSUM row 64 collects the softmax denominator.  M=65
    keeps LDWEIGHTS at 65 cols and needs no zero padding.
  - normalize: O^T copied to SBUF on gpsimd (frees the PSUM bank),
    exact DVE reciprocal on the [1,512] denominator row, ONE SBUF->SBUF
    broadcast dma [1,512]->[64,512], DVE multiply -> y^T bf16.  No DRAM
    round-trips.
  - c_proj per 128-row block: 4 matmuls -> bf16 SBUF -> one [128,1024]
    dma to HBM (bf16 partials, summed in f32 on host).
  - q-blocks processed 3,2,1,0 so the shortest dependency chain is last;
    proj work for block N drained during block N-1's attention.
"""

import sys
import types

import numpy as np

# ---------------------------------------------------------------------------
# Environment compatibility (self-contained on purpose).
# ---------------------------------------------------------------------------


def _install_axon_ntff_hook():
    """Provide the missing ``antenv.axon_hooks`` module so that
    ``run_bass_kernel_spmd(trace=True)`` works under axon in this container."""
    if "antenv.axon_hooks" in sys.modules:
        return
    try:
        import antenv
    except ImportError:
        return
    mod = types.ModuleType("antenv.axon_hooks")
    holder = [None]
    mod.set_axon_ntff_profile_hook = lambda h: holder.__setitem__(0, h)
    mod.get_axon_ntff_profile_hook = lambda: holder[0]
    sys.modules["antenv.axon_hooks"] = mod
    antenv.axon_hooks = mod
    try:
        from trn_agent_boot.trn_boot import _ntff_profile_via_ctypes

        hook = _ntff_profile_via_ctypes("/opt/axon/libaxon_pjrt.so")
        if hook is not None:
            mod.set_axon_ntff_profile_hook(hook)
    except Exception:
        pass


_install_axon_ntff_hook()

import concourse.bass as bass  # noqa: E402
import concourse.mybir as mybir  # noqa: E402
import concourse.tile as tile  # noqa: E402
from concourse.bass_utils import run_bass_kernel_spmd  # noqa: E402


def _split_multi_waits(nc, max_waits=1):
    """The walrus build here rejects instructions with more than one sync
    wait; move excess waits onto same-engine NoOps placed just before the
    instruction (sequential waiting is equivalent for monotonic sems)."""
    n = 0
    for func in nc.m.functions:
        for bb in func.blocks:
            out = []
            changed = False
            for inst in bb.instructions:
                si = inst.sync_info
                waits = list(si.on_wait) if si is not None and si.on_wait else []
                if len(waits) > max_waits:
                    changed = True
                    extra, keep = waits[:-max_waits], waits[-max_waits:]
                    for i in range(0, len(extra), max_waits):
                        n += 1
                        out.append(
                            mybir.InstNoOp(
                                name=f"{inst.name}-ws{i}",
                                engine=inst.engine,
                                ins=[],
                                outs=[],
                                sync_info=mybir.SyncInfo(
                                    on_wait=extra[i : i + max_waits], on_update=[]
                                ),
                                text_hint="wait_split",
                            )
                        )
                    si.on_wait = keep
                out.append(inst)
            if changed:
                bb.instructions = out
    return n


# ---------------------------------------------------------------------------
# Problem constants (hardcoded per spec).
# ---------------------------------------------------------------------------

B, T, C = 2, 2048, 1024
N_HEAD = 16
D = 64  # head dim
N_CORES = 8
HG = 4  # head groups (cores per batch element)
NH = N_HEAD // HG  # heads per core = 4
HP = NH // 2  # head pairs per core = 2
HD = NH * D  # head channels per core = 256
CK = C // 128  # contraction chunks = 8
TT = T // 128  # t tiles = 16
QB = T // 512  # q blocks = 4
NCH = 8  # x column chunks (256 cols each)

F32 = mybir.dt.float32
MM_DT = mybir.dt.bfloat16
MM_NP = mybir.dt.np(MM_DT)

TRACE = False
LAST_RESULT = None
_NC_CACHE = {}


def _build_nc():
    nc = bass.Bass("TRN2", target_bir_lowering=False)

    xh = nc.dram_tensor("xh", [128, NCH, CK, 256], MM_DT, kind="ExternalInput")
    wq = nc.dram_tensor("wq", [128, CK, HD], MM_DT, kind="ExternalInput")
    wk = nc.dram_tensor("wk", [128, CK, HD], MM_DT, kind="ExternalInput")
    wv = nc.dram_tensor("wv", [128, CK, HD], MM_DT, kind="ExternalInput")
    bias = nc.dram_tensor("bias", [128, 4], F32, kind="ExternalInput")
    wp = nc.dram_tensor("wp", [128, HD // 128, C], MM_DT, kind="ExternalInput")
    out = nc.dram_tensor("out", [T, C], MM_DT, kind="ExternalOutput")

    with tile.TileContext(nc) as tc:
        _emit(nc, tc, xh, wq, wk, wv, bias, wp, out)

    _split_multi_waits(nc)
    return nc


def _emit(nc, tc, xh, wq, wk, wv, bias, wp, out):
    from contextlib import ExitStack

    ctx = ExitStack()
    with ctx:
        consts = ctx.enter_context(tc.tile_pool(name="consts", bufs=1))
        qz_pool = ctx.enter_context(tc.tile_pool(name="qz", bufs=HP))
        kt_pool = ctx.enter_context(tc.tile_pool(name="kt", bufs=HD // 128))
        yt_pool = ctx.enter_context(tc.tile_pool(name="yt", bufs=2))
        pt_pool = ctx.enter_context(tc.tile_pool(name="pt", bufs=6))
        os_pool = ctx.enter_context(tc.tile_pool(name="os", bufs=4))
        rb_pool = ctx.enter_context(tc.tile_pool(name="rb", bufs=6))
        ob_pool = ctx.enter_context(tc.tile_pool(name="ob", bufs=3))
        dram = ctx.enter_context(tc.tile_pool(name="dram", bufs=3, space="DRAM"))
        # PSUM: st 2x2 banks + ot 2x1 + qk 2x1 = 8 banks
        p_qk = ctx.enter_context(tc.tile_pool(name="p_qk", bufs=2, space="PSUM"))
        p_st = ctx.enter_context(tc.tile_pool(name="p_st", bufs=2, space="PSUM"))
        p_ot = ctx.enter_context(tc.tile_pool(name="p_ot", bufs=2, space="PSUM"))

        # ---- loads ----------------------------------------------------------
        # x chunks on the sync queue; everything else on vector so issue
        # overlaps.  Each dma is one 128-descriptor post.
        bias_sb = consts.tile([128, 4], F32, tag="bias")
        nc.scalar.dma_start(bias_sb[:], bias[:])
        wv_t = consts.tile([128, CK, HD], MM_DT, tag="wv")
        nc.scalar.dma_start(wv_t[:], wv[:])
        wk_t = consts.tile([128, CK, HD], MM_DT, tag="wk")
        nc.scalar.dma_start(wk_t[:], wk[:])

        xt = consts.tile([128, NCH, CK, 256], MM_DT, tag="xt")
        for c in range(NCH):
            nc.sync.dma_start(xt[:, c], xh[:, c])

        wq_t = consts.tile([128, CK, HD], MM_DT, tag="wq")
        nc.scalar.dma_start(wq_t[:], wq[:])
        wp_t = consts.tile([128, HD // 128, C], MM_DT, tag="wp")
        nc.scalar.dma_start(wp_t[:], wp[:])

        # V tiles [128, tt, h, 65]: per head [V(64) | 1] so AV's psum row 64
        # accumulates the softmax denominator.  Ones column set once.
        vo = consts.tile([128, TT, NH, 65], MM_DT, tag="vo")
        nc.gpsimd.memset(
            vo[:].rearrange("p t h c -> p (t h) c")[:, :, D : D + 1], 1.0
        )

        # ---- QKV projection -------------------------------------------------
        qz_sb = [
            qz_pool.tile([128, T], MM_DT, tag="qz", name=f"qz{hp}")
            for hp in range(HP)
        ]
        kt_sb = [
            kt_pool.tile([128, T], MM_DT, tag="kt", name=f"kt{i}")
            for i in range(HD // 128)
        ]

        def emit_v(tt):
            c, half = tt // 2, tt % 2
            ps = p_qk.tile([128, 512], F32, tag="pq")
            for ck in range(CK):
                nc.tensor.matmul(
                    ps[:, :HD],
                    xt[:, c, ck, half * 128 : half * 128 + 128],
                    wv_t[:, ck, :],
                    start=(ck == 0),
                    stop=(ck == CK - 1),
                )
            nc.vector.tensor_copy(
                vo[:, tt, :, 0:D],
                ps[:, :HD].rearrange("p (h d) -> p h d", h=NH),
            )

        def emit_k(i, tb):
            ps = p_qk.tile([128, 512], F32, tag="pq")
            for ck in range(CK):
                nc.tensor.matmul(
                    ps[:],
                    wk_t[:, ck, i * 128 : (i + 1) * 128],
                    xt[:, 2 * tb : 2 * tb + 2, ck, :],
                    start=(ck == 0),
                    stop=(ck == CK - 1),
                )
            nc.vector.tensor_scalar(
                kt_sb[i][:, tb * 512 : (tb + 1) * 512],
                ps[:],
                bias_sb[:, 2 + i : 3 + i],
                None,
                mybir.AluOpType.add,
            )

        def emit_q(hp, tb):
            ps = p_qk.tile([128, 512], F32, tag="pq")
            for ck in range(CK):
                nc.tensor.matmul(
                    ps[:],
                    wq_t[:, ck, hp * 128 : (hp + 1) * 128],
                    xt[:, 2 * tb : 2 * tb + 2, ck, :],
                    start=(ck == 0),
                    stop=(ck == CK - 1),
                )
            nc.vector.tensor_scalar(
                qz_sb[hp][:, tb * 512 : (tb + 1) * 512],
                ps[:],
                bias_sb[:, hp : hp + 1],
                None,
                mybir.AluOpType.add,
            )

        # V first (each needs only one 128-col x slice), K interleaved as its
        # 512-col spans complete.
        for c in range(NCH):
            emit_v(2 * c)
            emit_v(2 * c + 1)
            if c % 2 == 1:
                tb = c // 2
                emit_k(0, tb)
                emit_k(1, tb)
        emit_q(0, 3)
        emit_q(1, 3)

        # ---- attention ------------------------------------------------------
        yt_sb = [
            yt_pool.tile([128, T], MM_DT, tag="yt", name=f"yt{g}")
            for g in range(HD // 128)
        ]

        # c_proj granules: one 128-row block of out per granule (4 matmuls,
        # 2 PSUM->SBUF bf16 copies, 1 dma); queued when a q-block's heads
        # finish, drained as PE fillers during the next q-block.
        proj_queue = []

        def make_proj(tt):
            def emit_proj():
                ob = ob_pool.tile([128, C], MM_DT, tag="ob")
                for nb in range(C // 512):
                    ps = p_qk.tile([128, 512], F32, tag="pq")
                    for g in range(HD // 128):
                        nc.tensor.matmul(
                            ps[:],
                            yt_sb[g][:, tt * 128 : (tt + 1) * 128],
                            wp_t[:, g, nb * 512 : (nb + 1) * 512],
                            start=(g == 0),
                            stop=(g == HD // 128 - 1),
                        )
                    nc.vector.tensor_copy(
                        ob[:, nb * 512 : (nb + 1) * 512], ps[:]
                    )
                nc.sync.dma_start(out[tt * 128 : (tt + 1) * 128, :], ob[:])

            return emit_proj

        def drain_proj(n):
            for _ in range(min(n, len(proj_queue))):
                proj_queue.pop(0)()

        def normalize(h, ot, q0):
            # O^T rows to SBUF (gpsimd; frees the PSUM bank), exact DVE
            # reciprocal of the denominator row, one SBUF->SBUF broadcast
            # dma, multiply -> y^T bf16.
            g, jb = h // 2, (h % 2) * 64
            o_sb = os_pool.tile([65, 512], F32, tag="os")
            nc.vector.tensor_copy(o_sb[:], ot[:])
            rc = rb_pool.tile([1, 512], F32, tag="rc")
            nc.vector.reciprocal(rc[:], o_sb[64:65, :])
            rc_d = dram.tile([1, 512], F32, tag="rc_d")
            nc.sync.dma_start(rc_d[:], rc[:])
            rb = rb_pool.tile([64, 512], F32, tag="rb")
            nc.sync.dma_start(rb[:], rc_d[:].to_broadcast((64, 512)))
            nc.vector.tensor_tensor(
                yt_sb[g][jb : jb + 64, q0 : q0 + 512],
                o_sb[0:64, :],
                rb[:],
                mybir.AluOpType.mult,
            )

        def emit_attention(qb):
            q0 = qb * 512
            n_kt = 4 * qb + 4

            for hp in range(HP):
                kd = kt_sb[hp]
                qd = qz_sb[hp]
                ots = {
                    jj: p_ot.tile([65, 512], F32, tag="ot", name=f"ot{hp}{jj}")
                    for jj in range(2)
                }

                def emit_av(kt, c, pt2):
                    for jj in range(2):
                        nc.tensor.matmul(
                            ots[jj][:, c:512],
                            vo[:, kt, 2 * hp + jj, :],
                            pt2[:, jj, c:512],
                            start=(kt == 0),
                            stop=(kt == n_kt - 1),
                        )

                pending = []
                for kt in range(n_kt):
                    j = kt - 4 * qb
                    c = 128 * j if j >= 0 else 0
                    # scores for BOTH heads: two K=64 matmuls on disjoint
                    # PE row-groups, executing concurrently.
                    st = p_st.tile([128, 1024], F32, tag="st")
                    st2 = st[:].rearrange("p (h q) -> p h q", h=2)
                    for jj in range(2):
                        nc.tensor.matmul(
                            st[:, jj * 512 + c : jj * 512 + 512],
                            kd[jj * 64 : jj * 64 + 64, kt * 128 : (kt + 1) * 128],
                            qd[jj * 64 : jj * 64 + 64, q0 + c : q0 + 512],
                            start=True,
                            stop=True,
                        )
                    pt = pt_pool.tile([128, 1024], MM_DT, tag="pt")
                    pt2 = pt[:].rearrange("p (h q) -> p h q", h=2)
                    nc.scalar.activation(
                        pt2[:, :, c:512],
                        st2[:, :, c:512],
                        mybir.ActivationFunctionType.Exp,
                        scale=0.125,
                    )
                    if j >= 0:
                        # causal mask: zero upper triangle of the diagonal
                        # 128-col window, both heads, after exp, on gpsimd.
                        nc.gpsimd.affine_select(
                            out=pt2[:, :, c : c + 128],
                            in_=pt2[:, :, c : c + 128],
                            compare_op=mybir.AluOpType.is_ge,
                            fill=0.0,
                            base=0,
                            pattern=[[0, 2], [1, 128]],
                            channel_multiplier=-1,
                        )
                    pending.append((kt, c, pt2))
                    if len(pending) > 1:
                        emit_av(*pending.pop(0))
                    if kt % 3 == 2:
                        drain_proj(1)
                for p in pending:
                    emit_av(*p)
                for jj in range(2):
                    normalize(2 * hp + jj, ots[jj], q0)
                if hp == 0 and qb > 0:
                    # Q projection for the next q-block: PE filler while
                    # the scalar engine chews exp.
                    emit_q(0, qb - 1)
                    emit_q(1, qb - 1)
                drain_proj(1)

            # queue this q-block's c_proj row-blocks (drained during the
            # next q-block's attention; leftovers drained after the loop).
            for tt in range(qb * 4, qb * 4 + 4):
                proj_queue.append(make_proj(tt))

        # q-blocks descending: longest k-chain first, shortest last (small
        # serial tail).
        for qb in (3, 2, 1, 0):
            emit_attention(qb)
        drain_proj(len(proj_queue))


def _get_nc():
    key = str(MM_DT)
    if key not in _NC_CACHE:
        _NC_CACHE[key] = _build_nc()
    return _NC_CACHE[key]


def kernel(x, Wqkv, bqkv, Wproj, bproj):
    global LAST_RESULT
    x = np.asarray(x, dtype=np.float32)
    Wqkv = np.asarray(Wqkv, dtype=np.float32)
    bqkv = np.asarray(bqkv, dtype=np.float32)
    Wproj = np.asarray(Wproj, dtype=np.float32)
    bproj = np.asarray(bproj, dtype=np.float32)

    nc = _get_nc()
    in_maps = []
    for core in range(N_CORES):
        b, hg = core // HG, core % HG
        cs, ce = hg * HD, (hg + 1) * HD
        # x chunk-major: [p, chunk, ck, 256]
        xT = x[b].T  # [C, T] = [(ck p), t]
        xh = np.ascontiguousarray(
            xT.reshape(CK, 128, NCH, 256).transpose(1, 2, 0, 3).astype(MM_NP)
        )
        # weights p-major: [p, ck, n]
        def wslice(w):
            return np.ascontiguousarray(
                w.reshape(CK, 128, HD).transpose(1, 0, 2).astype(MM_NP)
            )

        bq = bqkv[cs:ce].reshape(HP, 128).T  # [128, HP]
        bk = bqkv[C + cs : C + ce].reshape(2, 128).T  # [128, 2]
        bias = np.ascontiguousarray(
            np.concatenate([bq, bk], axis=1).astype(np.float32)
        )
        in_maps.append(
            {
                "xh": xh,
                "wq": wslice(Wqkv[:, cs:ce]),
                "wk": wslice(Wqkv[:, C + cs : C + ce]),
                "wv": wslice(Wqkv[:, 2 * C + cs : 2 * C + ce]),
                "bias": bias,
                "wp": np.ascontiguousarray(
                    Wproj[cs:ce, :]
                    .reshape(HD // 128, 128, C)
                    .transpose(1, 0, 2)
                    .astype(MM_NP)
                ),
            }
        )

    res = run_bass_kernel_spmd(
        nc, in_maps, core_ids=list(range(N_CORES)), trace=TRACE
    )
    LAST_RESULT = res

    # V-bias contribution: y_true = y_dev + bv per head concat, and softmax
    # rows sum to exactly 1, so out += bv @ Wproj (host-side, exact).
    bv_full = bqkv[2 * C : 3 * C]
    bias_term = bv_full @ Wproj + bproj

    outp = np.empty((B, T, C), dtype=np.float32)
    for b in range(B):
        acc = res.results[b * HG]["out"].astype(np.float32)
        for hg in range(1, HG):
            acc = acc + res.results[b * HG + hg]["out"].astype(np.float32)
        outp[b] = acc + bias_term
    return outp


# revision 12
# speedup vs baseline: 1.1680x; 1.0986x over previous
"""Causal self-attention (B=2, T=2048, C=1024, 16 heads) on 8 trn2 cores.

Sharding: core = (batch b, head-group hg) on a 2x4 grid; each core computes
QKV projection, causal attention and the partial c_proj for its 4 heads of
one batch element. Host sums the 4 partials per batch element (replaces the
all-reduce) and adds bproj + bv@Wproj (the V-bias contribution is exact
because softmax rows sum to 1).

Device layout per core (all matmuls bf16):
  - x arrives host-prearranged chunk-major as xh [128, chunk=8, ck=8, 256]
    so each 256-column chunk is ONE dma (128 descriptors) and the first
    projection matmuls start ~2us in.  Weights arrive p-major so each is a
    single 128-descriptor dma.  DMA issue is split across sync (x, out,
    broadcasts) and vector (weights) queues.
  - K^T / Q^T produced as [128, T] bf16 tiles holding a HEAD-PAIR: head
    2hp at partitions 0..63, head 2hp+1 at 64..127.  One [128,512]
    tensor_scalar adds the bias (per-partition AP) and casts to bf16.
  - Scores: per k-tile, BOTH heads of a pair via two K=64 matmuls on
    disjoint PE row-groups (tile_position (0,0) / (64,0)) into one
    [128,1024] PSUM pair; they execute concurrently on the 128x128 array.
  - exp: ONE ScalarE activation per k-tile over [128, 2, 512-c] (both
    heads, ragged diagonal offset c shared).  Causal mask applied AFTER
    exp by one gpsimd affine_select per diagonal k-tile (both heads).
  - V stored [128, tt, h, 65] = per head [V(64) | 1]; AV accumulates
    O^T[65, q] so PSUM row 64 collects the softmax denominator.  M=65
    keeps LDWEIGHTS at 65 cols and needs no zero padding.
  - normalize: O^T copied to SBUF on gpsimd (frees the PSUM bank),
    exact DVE reciprocal on the [1,512] denominator row, ONE SBUF->SBUF
    broadcast dma [1,512]->[64,512], DVE multiply -> y^T bf16.  No DRAM
    round-trips.
  - c_proj per 128-row block: 4 matmuls -> bf16 SBUF -> one [128,1024]
    dma to HBM (bf16 partials, summed in f32 on host).
  - q-blocks processed 3,2,1,0 so the shortest dependency chain is last;
    proj work for block N drained during block N-1's attention.
"""

import sys
import types

import numpy as np

# ---------------------------------------------------------------------------
# Environment compatibility (self-contained on purpose).
# ---------------------------------------------------------------------------


def _install_axon_ntff_hook():
    """Provide the missing ``antenv.axon_hooks`` module so that
    ``run_bass_kernel_spmd(trace=True)`` works under axon in this container."""
    if "antenv.axon_hooks" in sys.modules:
        return
    try:
        import antenv
    except ImportError:
        return
    mod = types.ModuleType("antenv.axon_hooks")
    holder = [None]
    mod.set_axon_ntff_profile_hook = lambda h: holder.__setitem__(0, h)
    mod.get_axon_ntff_profile_hook = lambda: holder[0]
    sys.modules["antenv.axon_hooks"] = mod
    antenv.axon_hooks = mod
    try:
        from trn_agent_boot.trn_boot import _ntff_profile_via_ctypes

        hook = _ntff_profile_via_ctypes("/opt/axon/libaxon_pjrt.so")
        if hook is not None:
            mod.set_axon_ntff_profile_hook(hook)
    except Exception:
        pass


_install_axon_ntff_hook()

import concourse.bass as bass  # noqa: E402
import concourse.mybir as mybir  # noqa: E402
import concourse.tile as tile  # noqa: E402
from concourse.bass_utils import run_bass_kernel_spmd  # noqa: E402


def _split_multi_waits(nc, max_waits=1):
    """The walrus build here rejects instructions with more than one sync
    wait; move excess waits onto same-engine NoOps placed just before the
    instruction (sequential waiting is equivalent for monotonic sems)."""
    n = 0
    for func in nc.m.functions:
        for bb in func.blocks:
            out = []
            changed = False
            for inst in bb.instructions:
                si = inst.sync_info
                waits = list(si.on_wait) if si is not None and si.on_wait else []
                if len(waits) > max_waits:
                    changed = True
                    extra, keep = waits[:-max_waits], waits[-max_waits:]
                    for i in range(0, len(extra), max_waits):
                        n += 1
                        out.append(
                            mybir.InstNoOp(
                                name=f"{inst.name}-ws{i}",
                                engine=inst.engine,
                                ins=[],
                                outs=[],
                                sync_info=mybir.SyncInfo(
                                    on_wait=extra[i : i + max_waits], on_update=[]
                                ),
                                text_hint="wait_split",
                            )
                        )
                    si.on_wait = keep
                out.append(inst)
            if changed:
                bb.instructions = out
    return n


# ---------------------------------------------------------------------------
# Problem constants (hardcoded per spec).
# ---------------------------------------------------------------------------

B, T, C = 2, 2048, 1024
N_HEAD = 16
D = 64  # head dim
N_CORES = 8
HG = 4  # head groups (cores per batch element)
NH = N_HEAD // HG  # heads per core = 4
HP = NH // 2  # head pairs per core = 2
HD = NH * D  # head channels per core = 256
CK = C // 128  # contraction chunks = 8
TT = T // 128  # t tiles = 16
QB = T // 512  # q blocks = 4
NCH = 8  # x column chunks (256 cols each)

F32 = mybir.dt.float32
MM_DT = mybir.dt.bfloat16
MM_NP = mybir.dt.np(MM_DT)

TRACE = False
LAST_RESULT = None
_NC_CACHE = {}


def _build_nc():
    nc = bass.Bass("TRN2", target_bir_lowering=False)

    xh = nc.dram_tensor("xh", [128, NCH, CK, 256], MM_DT, kind="ExternalInput")
    wq = nc.dram_tensor("wq", [128, CK, HD], MM_DT, kind="ExternalInput")
    wk = nc.dram_tensor("wk", [128, CK, HD], MM_DT, kind="ExternalInput")
    wv = nc.dram_tensor("wv", [128, CK, HD], MM_DT, kind="ExternalInput")
    bias = nc.dram_tensor("bias", [128, 4], F32, kind="ExternalInput")
    wp = nc.dram_tensor("wp", [128, HD // 128, C], MM_DT, kind="ExternalInput")
    out = nc.dram_tensor("out", [T, C], MM_DT, kind="ExternalOutput")

    with tile.TileContext(nc) as tc:
        _emit(nc, tc, xh, wq, wk, wv, bias, wp, out)

    _split_multi_waits(nc)
    return nc


def _emit(nc, tc, xh, wq, wk, wv, bias, wp, out):
    from contextlib import ExitStack

    ctx = ExitStack()
    with ctx:
        consts = ctx.enter_context(tc.tile_pool(name="consts", bufs=1))
        qz_pool = ctx.enter_context(tc.tile_pool(name="qz", bufs=HP))
        kt_pool = ctx.enter_context(tc.tile_pool(name="kt", bufs=HD // 128))
        yt_pool = ctx.enter_context(tc.tile_pool(name="yt", bufs=2))
        pt_pool = ctx.enter_context(tc.tile_pool(name="pt", bufs=6))
        os_pool = ctx.enter_context(tc.tile_pool(name="os", bufs=4))
        rb_pool = ctx.enter_context(tc.tile_pool(name="rb", bufs=6))
        ob_pool = ctx.enter_context(tc.tile_pool(name="ob", bufs=3))
        dram = ctx.enter_context(tc.tile_pool(name="dram", bufs=3, space="DRAM"))
        # PSUM: st 2x2 banks + ot 2x1 + qk 2x1 = 8 banks
        p_qk = ctx.enter_context(tc.tile_pool(name="p_qk", bufs=2, space="PSUM"))
        p_st = ctx.enter_context(tc.tile_pool(name="p_st", bufs=2, space="PSUM"))
        p_ot = ctx.enter_context(tc.tile_pool(name="p_ot", bufs=2, space="PSUM"))

        # ---- loads ----------------------------------------------------------
        # x chunks on the sync queue; weights split in halves across the
        # scalar and gpsimd queues so the first K/V matmuls only wait for
        # ~256KB.  Each dma is a <=128-descriptor post.
        wk_t = consts.tile([128, CK, HD], MM_DT, tag="wk")
        nc.scalar.dma_start(wk_t[:, 0:4], wk[:, 0:4])
        wv_t = consts.tile([128, CK, HD], MM_DT, tag="wv")
        nc.gpsimd.dma_start(wv_t[:, 0:4], wv[:, 0:4])
        nc.scalar.dma_start(wk_t[:, 4:8], wk[:, 4:8])
        nc.gpsimd.dma_start(wv_t[:, 4:8], wv[:, 4:8])
        bias_sb = consts.tile([128, 4], F32, tag="bias")
        nc.scalar.dma_start(bias_sb[:], bias[:])

        xt = consts.tile([128, NCH, CK, 256], MM_DT, tag="xt")
        for c in range(NCH):
            nc.sync.dma_start(xt[:, c], xh[:, c])

        wq_t = consts.tile([128, CK, HD], MM_DT, tag="wq")
        nc.scalar.dma_start(wq_t[:, 0:4], wq[:, 0:4])
        nc.gpsimd.dma_start(wq_t[:, 4:8], wq[:, 4:8])
        wp_t = consts.tile([128, HD // 128, C], MM_DT, tag="wp")
        nc.scalar.dma_start(wp_t[:], wp[:])

        # V tiles [128, tt, h, 65]: per head [V(64) | 1] so AV's psum row 64
        # accumulates the softmax denominator.  Ones column set once.
        vo = consts.tile([128, TT, NH, 65], MM_DT, tag="vo")
        nc.gpsimd.memset(
            vo[:].rearrange("p t h c -> p (t h) c")[:, :, D : D + 1], 1.0
        )

        # ---- QKV projection -------------------------------------------------
        qz_sb = [
            qz_pool.tile([128, T], MM_DT, tag="qz", name=f"qz{hp}")
            for hp in range(HP)
        ]
        kt_sb = [
            kt_pool.tile([128, T], MM_DT, tag="kt", name=f"kt{i}")
            for i in range(HD // 128)
        ]

        def emit_v(tt):
            c, half = tt // 2, tt % 2
            ps = p_qk.tile([128, 512], F32, tag="pq")
            for ck in range(CK):
                nc.tensor.matmul(
                    ps[:, :HD],
                    xt[:, c, ck, half * 128 : half * 128 + 128],
                    wv_t[:, ck, :],
                    start=(ck == 0),
                    stop=(ck == CK - 1),
                )
            nc.vector.tensor_copy(
                vo[:, tt, :, 0:D],
                ps[:, :HD].rearrange("p (h d) -> p h d", h=NH),
            )

        def emit_k(i, tb):
            ps = p_qk.tile([128, 512], F32, tag="pq")
            for ck in range(CK):
                nc.tensor.matmul(
                    ps[:],
                    wk_t[:, ck, i * 128 : (i + 1) * 128],
                    xt[:, 2 * tb : 2 * tb + 2, ck, :],
                    start=(ck == 0),
                    stop=(ck == CK - 1),
                )
            nc.vector.tensor_scalar(
                kt_sb[i][:, tb * 512 : (tb + 1) * 512],
                ps[:],
                bias_sb[:, 2 + i : 3 + i],
                None,
                mybir.AluOpType.add,
            )

        def emit_q(hp, tb):
            ps = p_qk.tile([128, 512], F32, tag="pq")
            for ck in range(CK):
                nc.tensor.matmul(
                    ps[:],
                    wq_t[:, ck, hp * 128 : (hp + 1) * 128],
                    xt[:, 2 * tb : 2 * tb + 2, ck, :],
                    start=(ck == 0),
                    stop=(ck == CK - 1),
                )
            nc.vector.tensor_scalar(
                qz_sb[hp][:, tb * 512 : (tb + 1) * 512],
                ps[:],
                bias_sb[:, hp : hp + 1],
                None,
                mybir.AluOpType.add,
            )

        # K first as each 512-col span completes (the first q-block's scores
        # need all of K), V interleaved; the last V tiles and Q(qb=3) go
        # right before attention so exp starts as early as possible.
        for c in range(6):
            if c % 2 == 1:
                tb = c // 2
                emit_k(0, tb)
                emit_k(1, tb)
            emit_v(2 * c)
            emit_v(2 * c + 1)
        emit_k(0, 3)
        emit_k(1, 3)
        emit_q(0, 3)
        emit_q(1, 3)
        for tt in range(12, 16):
            emit_v(tt)

        # ---- attention ------------------------------------------------------
        yt_sb = [
            yt_pool.tile([128, T], MM_DT, tag="yt", name=f"yt{g}")
            for g in range(HD // 128)
        ]

        # c_proj granules: one 128-row block of out per granule (4 matmuls,
        # 2 PSUM->SBUF bf16 copies, 1 dma); queued when a q-block's heads
        # finish, drained as PE fillers during the next q-block.
        proj_queue = []

        def make_proj(tt):
            def emit_proj():
                ob = ob_pool.tile([128, C], MM_DT, tag="ob")
                for nb in range(C // 512):
                    ps = p_qk.tile([128, 512], F32, tag="pq")
                    for g in range(HD // 128):
                        nc.tensor.matmul(
                            ps[:],
                            yt_sb[g][:, tt * 128 : (tt + 1) * 128],
                            wp_t[:, g, nb * 512 : (nb + 1) * 512],
                            start=(g == 0),
                            stop=(g == HD // 128 - 1),
                        )
                    nc.vector.tensor_copy(
                        ob[:, nb * 512 : (nb + 1) * 512], ps[:]
                    )
                nc.sync.dma_start(out[tt * 128 : (tt + 1) * 128, :], ob[:])

            return emit_proj

        def drain_proj(n):
            for _ in range(min(n, len(proj_queue))):
                proj_queue.pop(0)()

        def normalize_a(ot):
            # O^T rows to SBUF (frees the PSUM bank for the next head pair).
            o_sb = os_pool.tile([65, 512], F32, tag="os")
            nc.vector.tensor_copy(o_sb[:], ot[:])
            return o_sb

        def normalize_b(h, o_sb, q0):
            # denominator folded [1,512] -> [128,4] via DRAM (DVE reciprocal
            # is an 8-pass iterative op: [1,512] on one lane costs 3.3us,
            # [128,4] costs 0.17us), exact reciprocal, broadcast back via
            # DRAM; y^T = O^T * r in bf16.
            g, jb = h // 2, (h % 2) * 64
            rc_d = dram.tile([1, 512], F32, tag="rc_d")
            nc.sync.dma_start(rc_d[:], o_sb[64:65, :])
            r4 = rb_pool.tile([128, 4], F32, tag="r4")
            nc.sync.dma_start(r4[:], rc_d[0, :].rearrange("(p o) -> p o", p=128))
            nc.vector.reciprocal(r4[:], r4[:])
            rc2_d = dram.tile([1, 512], F32, tag="rc2_d")
            nc.sync.dma_start(rc2_d[0, :].rearrange("(p o) -> p o", p=128), r4[:])
            rb = rb_pool.tile([64, 512], F32, tag="rb")
            nc.sync.dma_start(rb[:], rc2_d[:].to_broadcast((64, 512)))
            nc.vector.tensor_tensor(
                yt_sb[g][jb : jb + 64, q0 : q0 + 512],
                o_sb[0:64, :],
                rb[:],
                mybir.AluOpType.mult,
            )

        def emit_attention(qb):
            q0 = qb * 512
            n_kt = 4 * qb + 4

            for hp in range(HP):
                kd = kt_sb[hp]
                qd = qz_sb[hp]
                ots = {
                    jj: p_ot.tile([65, 512], F32, tag="ot", name=f"ot{hp}{jj}")
                    for jj in range(2)
                }

                def emit_av(kt, c, pt2):
                    for jj in range(2):
                        nc.tensor.matmul(
                            ots[jj][:, c:512],
                            vo[:, kt, 2 * hp + jj, :],
                            pt2[:, jj, c:512],
                            start=(kt == 0),
                            stop=(kt == n_kt - 1),
                        )

                pending = []
                for kt in range(n_kt):
                    j = kt - 4 * qb
                    c = 128 * j if j >= 0 else 0
                    # scores for BOTH heads: two K=64 matmuls on disjoint
                    # PE row-groups, executing concurrently.
                    st = p_st.tile([128, 1024], F32, tag="st")
                    st2 = st[:].rearrange("p (h q) -> p h q", h=2)
                    for jj in range(2):
                        nc.tensor.matmul(
                            st[:, jj * 512 + c : jj * 512 + 512],
                            kd[jj * 64 : jj * 64 + 64, kt * 128 : (kt + 1) * 128],
                            qd[jj * 64 : jj * 64 + 64, q0 + c : q0 + 512],
                            start=True,
                            stop=True,
                        )
                    pt = pt_pool.tile([128, 1024], MM_DT, tag="pt")
                    pt2 = pt[:].rearrange("p (h q) -> p h q", h=2)
                    nc.scalar.activation(
                        pt2[:, :, c:512],
                        st2[:, :, c:512],
                        mybir.ActivationFunctionType.Exp,
                        scale=0.125,
                    )
                    if j >= 0:
                        # causal mask: zero upper triangle of the diagonal
                        # 128-col window, both heads, after exp, on gpsimd.
                        nc.gpsimd.affine_select(
                            out=pt2[:, :, c : c + 128],
                            in_=pt2[:, :, c : c + 128],
                            compare_op=mybir.AluOpType.is_ge,
                            fill=0.0,
                            base=0,
                            pattern=[[0, 2], [1, 128]],
                            channel_multiplier=-1,
                        )
                    pending.append((kt, c, pt2))
                    if len(pending) > 1:
                        emit_av(*pending.pop(0))
                    if kt % 2 == 1:
                        drain_proj(1)
                for p in pending:
                    emit_av(*p)
                o_sbs = [normalize_a(ots[jj]) for jj in range(2)]
                for jj in range(2):
                    normalize_b(2 * hp + jj, o_sbs[jj], q0)
                if hp == 0 and qb > 0:
                    # Q projection for the next q-block: PE filler while
                    # the scalar engine chews exp.
                    emit_q(0, qb - 1)
                    emit_q(1, qb - 1)
                drain_proj(1)

            # queue this q-block's c_proj row-blocks (drained during the
            # next q-block's attention; leftovers drained after the loop).
            for tt in range(qb * 4, qb * 4 + 4):
                proj_queue.append(make_proj(tt))

        # q-blocks descending: longest k-chain first, shortest last (small
        # serial tail).
        for qb in (3, 2, 1, 0):
            emit_attention(qb)
        drain_proj(len(proj_queue))


def _get_nc():
    key = str(MM_DT)
    if key not in _NC_CACHE:
        _NC_CACHE[key] = _build_nc()
    return _NC_CACHE[key]


def kernel(x, Wqkv, bqkv, Wproj, bproj):
    global LAST_RESULT
    x = np.asarray(x, dtype=np.float32)
    Wqkv = np.asarray(Wqkv, dtype=np.float32)
    bqkv = np.asarray(bqkv, dtype=np.float32)
    Wproj = np.asarray(Wproj, dtype=np.float32)
    bproj = np.asarray(bproj, dtype=np.float32)

    nc = _get_nc()
    in_maps = []
    for core in range(N_CORES):
        b, hg = core // HG, core % HG
        cs, ce = hg * HD, (hg + 1) * HD
        # x chunk-major: [p, chunk, ck, 256]
        xT = x[b].T  # [C, T] = [(ck p), t]
        xh = np.ascontiguousarray(
            xT.reshape(CK, 128, NCH, 256).transpose(1, 2, 0, 3).astype(MM_NP)
        )
        # weights p-major: [p, ck, n]
        def wslice(w):
            return np.ascontiguousarray(
                w.reshape(CK, 128, HD).transpose(1, 0, 2).astype(MM_NP)
            )

        bq = bqkv[cs:ce].reshape(HP, 128).T  # [128, HP]
        bk = bqkv[C + cs : C + ce].reshape(2, 128).T  # [128, 2]
        bias = np.ascontiguousarray(
            np.concatenate([bq, bk], axis=1).astype(np.float32)
        )
        in_maps.append(
            {
                "xh": xh,
                "wq": wslice(Wqkv[:, cs:ce]),
                "wk": wslice(Wqkv[:, C + cs : C + ce]),
                "wv": wslice(Wqkv[:, 2 * C + cs : 2 * C + ce]),
                "bias": bias,
                "wp": np.ascontiguousarray(
                    Wproj[cs:ce, :]
                    .reshape(HD // 128, 128, C)
                    .transpose(1, 0, 2)
                    .astype(MM_NP)
                ),
            }
        )

    res = run_bass_kernel_spmd(
        nc, in_maps, core_ids=list(range(N_CORES)), trace=TRACE
    )
    LAST_RESULT = res

    # V-bias contribution: y_true = y_dev + bv per head concat, and softmax
    # rows sum to exactly 1, so out += bv @ Wproj (host-side, exact).
    bv_full = bqkv[2 * C : 3 * C]
    bias_term = bv_full @ Wproj + bproj

    outp = np.empty((B, T, C), dtype=np.float32)
    for b in range(B):
        acc = res.results[b * HG]["out"].astype(np.float32)
        for hg in range(1, HG):
            acc = acc + res.results[b * HG + hg]["out"].astype(np.float32)
        outp[b] = acc + bias_term
    return outp
